# revision 6
# baseline (speedup 1.0000x reference)
"""BitLinear (sign*scale weights) y = x @ (signs*scales).T on 8 trn2 cores.

Column-parallel: signs/scales sharded along out_features (11008 -> 8x1376),
x replicated, outputs concatenated along out dim.

Per-core program:
  - DMA signs shard in 128-row blocks, casting int32->bf16 in the DMA (SWDGE).
  - Dequant: per 128-wide k-group, tensor_scalar_mul by per-partition scale.
  - PE-transpose each [128,128] tile via identity into PSUM, copy to resident
    wT [k=128, groups=32, o=1376] bf16 in SBUF (DVE/ACT alternate).
  - x: load fp32, cast bf16, PE-transpose into xT [128, 32, 32].
  - Matmul: for each 512-wide o-chunk, accumulate 32 k-group matmuls in PSUM
    (bf16 inputs, fp32 accum), copy out, DMA y [32, 1376] fp32.
"""

import numpy as np

BATCH = 32
IN_F = 4096
OUT_F = 11008
GROUP = 128
N_GROUPS = IN_F // GROUP  # 32
N_CORES = 8
O_SHARD = OUT_F // N_CORES  # 1376
N_BLOCKS = (O_SHARD + 127) // 128  # 11 (10 full + 96-row remainder)
O_CHUNK = 512
CHUNK_BLOCKS = O_CHUNK // 128  # 4 blocks per matmul chunk

_nc_cache = []


def build_nc():
    import concourse.bacc as bacc
    import concourse.mybir as mybir
    import concourse.tile as tile
    from concourse.masks import make_identity

    f32 = mybir.dt.float32
    bf16 = mybir.dt.bfloat16
    i32 = mybir.dt.int32

    # Bacc (not Bass): its compile() runs generate_event_semaphores, which
    # splits multi-sem waits — walrus rejects >1 wait per instruction.
    nc = bacc.Bacc(None, target_bir_lowering=False)
    x_d = nc.dram_tensor("x", [BATCH, IN_F], f32, kind="ExternalInput")
    signs_d = nc.dram_tensor("signs", [O_SHARD, IN_F], i32, kind="ExternalInput")
    scales_d = nc.dram_tensor("scales", [O_SHARD, N_GROUPS], f32, kind="ExternalInput")
    y_d = nc.dram_tensor("y", [BATCH, O_SHARD], f32, kind="ExternalOutput")

    with tile.TileContext(nc) as tc:
        with tc.tile_pool(name="const", bufs=1) as const, tc.tile_pool(
            name="signs_p", bufs=2
        ) as signs_p, tc.tile_pool(name="w_p", bufs=2) as w_p, tc.tile_pool(
            name="psum", bufs=1, space="PSUM"
        ) as psum:
            ident = const.tile([128, 128], bf16, tag="ident")
            make_identity(nc, ident)

            x_sb = const.tile([BATCH, IN_F], f32, tag="x_sb")
            x_bf = const.tile([BATCH, IN_F], bf16, tag="x_bf")
            xT = const.tile([128, N_GROUPS, BATCH], bf16, tag="xT")
            scales_sb = const.tile([128, N_BLOCKS, N_GROUPS], f32, tag="scales_sb")
            wT = const.tile([128, N_GROUPS, O_SHARD], bf16, tag="wT")
            y_sb = const.tile([BATCH, O_SHARD], f32, tag="y_sb")

            # --- x prep: load, cast, transpose ---
            nc.sync.dma_start(x_sb[:], x_d[:])
            nc.vector.tensor_copy(x_bf[:], x_sb[:])
            for half in range(2):
                xp = psum.tile([128, 16, BATCH], bf16, tag="tp", bufs=4)
                for c in range(16):
                    j = half * 16 + c
                    nc.tensor.transpose(
                        xp[:, c, :],
                        x_bf[:, j * GROUP : (j + 1) * GROUP],
                        ident[:BATCH, :BATCH],
                    )
                nc.vector.tensor_copy(
                    xT[:, half * 16 : (half + 1) * 16, :], xp[:]
                )

            # --- per-block: DMA(cast) -> dequant -> transpose -> wT ---
            copy_engines = [nc.vector, nc.scalar]
            n_chunks = (O_SHARD + O_CHUNK - 1) // O_CHUNK  # 3
            y_tiles = {}

            for b in range(N_BLOCKS):
                r = min(128, O_SHARD - b * 128)
                nc.sync.dma_start(
                    scales_sb[:r, b, :], scales_d[b * 128 : b * 128 + r, :]
                )
                signs_bf = signs_p.tile([128, IN_F], bf16, tag="signs")
                nc.gpsimd.dma_start(
                    signs_bf[:r, :], signs_d[b * 128 : b * 128 + r, :]
                )
                w_bf = w_p.tile([128, IN_F], bf16, tag="w")
                for g in range(N_GROUPS):
                    # plain TensorTensor (broadcast AP) — TensorScalarPtr hits
                    # walrus "Too many sync wait commands" when carrying >=2 waits
                    nc.vector.tensor_tensor(
                        w_bf[:r, g * GROUP : (g + 1) * GROUP],
                        signs_bf[:r, g * GROUP : (g + 1) * GROUP],
                        scales_sb[:r, b, g : g + 1].to_broadcast([r, GROUP]),
                        mybir.AluOpType.mult,
                    )
                for t in range(8):  # 4 groups per psum tile
                    tp = psum.tile([128, 4, 128], bf16, tag="tp", bufs=4)
                    for c in range(4):
                        g = t * 4 + c
                        nc.tensor.transpose(
                            tp[:, c, :r],
                            w_bf[:r, g * GROUP : (g + 1) * GROUP],
                            ident[:r, :r],
                        )
                    eng = copy_engines[t % 2]
                    if eng is nc.scalar:
                        eng.copy(
                            wT[:, t * 4 : (t + 1) * 4, b * 128 : b * 128 + r],
                            tp[:, :, :r],
                        )
                    else:
                        eng.tensor_copy(
                            wT[:, t * 4 : (t + 1) * 4, b * 128 : b * 128 + r],
                            tp[:, :, :r],
                        )

                # --- matmuls for any o-chunk whose blocks are now complete ---
                for ci in range(n_chunks):
                    last_block = min((ci + 1) * CHUNK_BLOCKS, N_BLOCKS) - 1
                    if b != last_block:
                        continue
                    o0 = ci * O_CHUNK
                    w_o = min(O_CHUNK, O_SHARD - o0)
                    y_ps = psum.tile([BATCH, O_CHUNK], f32, tag=f"y{ci}", bufs=1)
                    y_tiles[ci] = (y_ps, o0, w_o)
                    for j in range(N_GROUPS):
                        nc.tensor.matmul(
                            y_ps[:, :w_o],
                            xT[:, j, :],
                            wT[:, j, o0 : o0 + w_o],
                            start=(j == 0),
                            stop=(j == N_GROUPS - 1),
                        )
                    nc.vector.tensor_copy(
                        y_sb[:, o0 : o0 + w_o], y_ps[:, :w_o]
                    )

            nc.sync.dma_start(y_d[:], y_sb[:])
    nc.finalize()
    return nc


def _shard_inputs(x, scales, signs):
    scales_r = scales.reshape(OUT_F, N_GROUPS)
    in_maps = []
    for c in range(N_CORES):
        lo, hi = c * O_SHARD, (c + 1) * O_SHARD
        in_maps.append(
            {
                "x": np.ascontiguousarray(x, dtype=np.float32),
                "signs": np.ascontiguousarray(signs[lo:hi], dtype=np.int32),
                "scales": np.ascontiguousarray(scales_r[lo:hi], dtype=np.float32),
            }
        )
    return in_maps


def _run(x, scales, signs, trace=False, tmpdir=None):
    from concourse import bass_utils

    if not _nc_cache:
        _nc_cache.append(build_nc())
    nc = _nc_cache[0]
    in_maps = _shard_inputs(x, scales, signs)
    res = bass_utils.run_bass_kernel_spmd(
        nc, in_maps, list(range(N_CORES)), trace=trace, tmpdir=tmpdir
    )
    out = np.concatenate(
        [np.asarray(res.results[i]["y"]) for i in range(N_CORES)], axis=1
    )
    return out.astype(np.float32), res


def kernel(x, scales, signs):
    out, _ = _run(x, scales, signs)
    return out


# revision 7
# speedup vs baseline: 1.6115x; 1.6115x over previous
"""BitLinear v3: transpose-free. Host ships signsT (pre-transposed, fp8 +/-1),
device does orientation-B matmuls (lhsT = raw sign tile, rhs = xT) into PSUM
partials per (block, group), dequant happens on the PSUM drain:
  yT[o,b] = sum_g scale[o,g] * (sT_g.T @ xT_g)[o,b]
Per block: 32 matmuls -> psum [r,16,32] x2, TT-mult by scale (free-broadcast
over b), one 4D strided reduce over (half, g) -> y_sb[r, b, 32].
Output is yT [1376, 32] per core; host transposes and concatenates.
"""

import numpy as np

BATCH = 32
IN_F = 4096
OUT_F = 11008
GROUP = 128
N_GROUPS = IN_F // GROUP  # 32
N_CORES = 8
O_SHARD = OUT_F // N_CORES  # 1376
N_BLOCKS = (O_SHARD + 127) // 128  # 11 (10 full + 96 remainder)
CHUNK_O = 256  # o-columns per DMA chunk (2 blocks)
N_CHUNKS = (O_SHARD + CHUNK_O - 1) // CHUNK_O  # 6 (last = 96 wide)

SIGN_DT = "fp8"  # "fp8" | "bf16"

_nc_cache = []


def build_nc():
    import concourse.bacc as bacc
    import concourse.mybir as mybir
    import concourse.tile as tile
    from concourse.masks import make_identity

    f32 = mybir.dt.float32
    bf16 = mybir.dt.bfloat16
    sdt = mybir.dt.float8e4 if SIGN_DT == "fp8" else bf16

    nc = bacc.Bacc(None, target_bir_lowering=False)
    x_d = nc.dram_tensor("x", [BATCH, IN_F], f32, kind="ExternalInput")
    sT_d = nc.dram_tensor("signsT", [IN_F, O_SHARD], sdt, kind="ExternalInput")
    scales_d = nc.dram_tensor("scales", [O_SHARD, N_GROUPS], f32, kind="ExternalInput")
    y_d = nc.dram_tensor("y", [O_SHARD, BATCH], f32, kind="ExternalOutput")

    with tile.TileContext(nc) as tc:
        with tc.tile_pool(name="const", bufs=1) as const, tc.tile_pool(
            name="tmp_p", bufs=2
        ) as tmp_p, tc.tile_pool(name="psum", bufs=1, space="PSUM") as psum:
            ident = const.tile([128, 128], bf16, tag="ident")
            make_identity(nc, ident)

            x_sb = const.tile([BATCH, IN_F], f32, tag="x_sb")
            x_bf = const.tile([BATCH, IN_F], bf16, tag="x_bf")
            xT = const.tile([128, N_GROUPS, BATCH], bf16, tag="xT")
            scales_sb = const.tile([128, N_BLOCKS, N_GROUPS], f32, tag="scales_sb")
            y_sb = const.tile([128, N_BLOCKS, BATCH], f32, tag="y_sb")

            # --- x prep: load, cast, transpose (PE) ---
            nc.sync.dma_start(x_sb[:], x_d[:])
            nc.vector.tensor_copy(x_bf[:], x_sb[:])
            for half in range(2):
                xp = psum.tile([128, 16, BATCH], bf16, tag="xp", bufs=2)
                for c in range(16):
                    g = half * 16 + c
                    nc.tensor.transpose(
                        xp[:, c, :],
                        x_bf[:, g * GROUP : (g + 1) * GROUP],
                        ident[:BATCH, :BATCH],
                    )
                nc.vector.tensor_copy(xT[:, half * 16 : (half + 1) * 16, :], xp[:])

            # --- signsT chunk DMAs: [128(k), 32(g), w(o)] per chunk ---
            sT_view = sT_d[:].rearrange("(g p) o -> p g o", p=128)
            s_chunks = []
            for c in range(N_CHUNKS):
                o0 = c * CHUNK_O
                w = min(CHUNK_O, O_SHARD - o0)
                sc = const.tile([128, N_GROUPS, w], sdt, tag=f"sT{c}")
                for q in range(4):  # split across queues
                    nc.sync.dma_start(
                        sc[:, q * 8 : (q + 1) * 8, :],
                        sT_view[:, q * 8 : (q + 1) * 8, o0 : o0 + w],
                    )
                s_chunks.append(sc)

            # --- scales ---
            for b in range(N_BLOCKS):
                r = min(128, O_SHARD - b * 128)
                nc.sync.dma_start(
                    scales_sb[:r, b, :], scales_d[b * 128 : b * 128 + r, :]
                )

            # --- per block: 32 matmuls -> scale-drain -> reduce ---
            for b in range(N_BLOCKS):
                r = min(128, O_SHARD - b * 128)
                sc = s_chunks[b // 2]
                oc = (b % 2) * 128
                psA = psum.tile([128, 16, BATCH], f32, tag="psA", bufs=2)
                psB = psum.tile([128, 16, BATCH], f32, tag="psB", bufs=2)
                for g in range(N_GROUPS):
                    ps = psA if g < 16 else psB
                    nc.tensor.matmul(
                        ps[:r, g % 16, :],
                        sc[:, g, oc : oc + r],
                        xT[:, g, :],
                        start=True,
                        stop=True,
                    )
                tmp = tmp_p.tile([128, 2, 16, BATCH], f32, tag="tmp")
                eng = nc.vector  # gpsimd cannot read PSUM
                eng.tensor_tensor(
                    tmp[:r, 0],
                    psA[:r],
                    scales_sb[:r, b, 0:16].to_broadcast([r, 16, BATCH]),
                    mybir.AluOpType.mult,
                )
                eng.tensor_tensor(
                    tmp[:r, 1],
                    psB[:r],
                    scales_sb[:r, b, 16:32].to_broadcast([r, 16, BATCH]),
                    mybir.AluOpType.mult,
                )
                nc.vector.tensor_reduce(
                    y_sb[:r, b, :],
                    tmp[:r].transpose([0, 3, 1, 2]),
                    axis=mybir.AxisListType.XY,
                    op=mybir.AluOpType.add,
                )

            # --- y out: [128, 11, 32] -> yT [1376, 32] ---
            nc.sync.dma_start(
                y_d[0 : 10 * 128].rearrange("(blk p) b -> p blk b", p=128),
                y_sb[:, 0:10, :],
            )
            nc.sync.dma_start(y_d[10 * 128 : O_SHARD], y_sb[:96, 10, :])
    nc.finalize()
    return nc


def _pack_signs(signs_shard):
    import ml_dtypes

    if SIGN_DT == "fp8":
        return np.ascontiguousarray(signs_shard.T).astype(np.float32).astype(
            ml_dtypes.float8_e4m3fn
        )
    return np.ascontiguousarray(signs_shard.T).astype(np.float32).astype(
        ml_dtypes.bfloat16
    )


def _shard_inputs(x, scales, signs):
    scales_r = scales.reshape(OUT_F, N_GROUPS)
    x32 = np.ascontiguousarray(x, dtype=np.float32)
    in_maps = []
    for c in range(N_CORES):
        lo, hi = c * O_SHARD, (c + 1) * O_SHARD
        in_maps.append(
            {
                "x": x32,
                "signsT": _pack_signs(signs[lo:hi]),
                "scales": np.ascontiguousarray(scales_r[lo:hi], dtype=np.float32),
            }
        )
    return in_maps


def _run(x, scales, signs, trace=False, tmpdir=None):
    from concourse import bass_utils

    if not _nc_cache:
        _nc_cache.append(build_nc())
    nc = _nc_cache[0]
    in_maps = _shard_inputs(x, scales, signs)
    res = bass_utils.run_bass_kernel_spmd(
        nc, in_maps, list(range(N_CORES)), trace=trace, tmpdir=tmpdir
    )
    out = np.concatenate(
        [np.asarray(res.results[i]["y"]).T for i in range(N_CORES)], axis=1
    )
    return np.ascontiguousarray(out).astype(np.float32), res


def kernel(x, scales, signs):
    out, _ = _run(x, scales, signs)
    return out


# revision 9
# speedup vs baseline: 1.9196x; 1.1911x over previous
"""BitLinear v3: transpose-free. Host ships signsT pre-transposed AND pre-packed
into the exact SBUF image (fp8 +/-1, per-chunk contiguous per partition) so DMA
descriptors are large. Device does orientation-B matmuls (lhsT = raw sign tile,
rhs = xT) into a b-major PSUM tile via strided writes; dequant+reduce on drain:
  yT[o,b] = sum_g scale[o,g] * (sT_g.T @ xT_g)[o,b]
Per block: 32 matmuls -> psum [r,32b,32g], one TT-mult by scale (middle
broadcast over b), one contiguous innermost reduce over g -> y_sb[r, b, :].
Output is yT [1376, 32] per core; host transposes and concatenates.
"""

import numpy as np

BATCH = 32
IN_F = 4096
OUT_F = 11008
GROUP = 128
N_GROUPS = IN_F // GROUP  # 32
N_CORES = 8
O_SHARD = OUT_F // N_CORES  # 1376
N_BLOCKS = (O_SHARD + 127) // 128  # 11 (10 full + 96 remainder)
CHUNK_O = 256  # o-columns per DMA chunk (2 blocks)
N_CHUNKS = (O_SHARD + CHUNK_O - 1) // CHUNK_O  # 6 (last = 96 wide)
IMG_F = N_GROUPS * O_SHARD  # 44032 free bytes per partition (fp8)

SIGN_DT = "fp8"  # "fp8" | "bf16"

_nc_cache = []


def _chunk_widths():
    return [min(CHUNK_O, O_SHARD - c * CHUNK_O) for c in range(N_CHUNKS)]


def build_nc():
    import concourse.bacc as bacc
    import concourse.mybir as mybir
    import concourse.tile as tile
    from concourse.masks import make_identity

    f32 = mybir.dt.float32
    bf16 = mybir.dt.bfloat16
    sdt = mybir.dt.float8e4 if SIGN_DT == "fp8" else bf16

    nc = bacc.Bacc(None, target_bir_lowering=False)
    x_d = nc.dram_tensor("x", [BATCH, IN_F], f32, kind="ExternalInput")
    sT_d = nc.dram_tensor("signsT", [128, IMG_F], sdt, kind="ExternalInput")
    scales_d = nc.dram_tensor("scales", [O_SHARD, N_GROUPS], f32, kind="ExternalInput")
    y_d = nc.dram_tensor("y", [O_SHARD, BATCH], f32, kind="ExternalOutput")

    with tile.TileContext(nc) as tc:
        with tc.tile_pool(name="const", bufs=1) as const, tc.tile_pool(
            name="tmp_p", bufs=2
        ) as tmp_p, tc.tile_pool(name="psum", bufs=1, space="PSUM") as psum:
            ident = const.tile([128, 128], bf16, tag="ident")
            make_identity(nc, ident)

            x_sb = const.tile([BATCH, IN_F], f32, tag="x_sb")
            x_bf = const.tile([BATCH, IN_F], bf16, tag="x_bf")
            xT = const.tile([128, N_GROUPS, BATCH], bf16, tag="xT")
            scales_sb = const.tile([128, N_BLOCKS, N_GROUPS], f32, tag="scales_sb")
            y_sb = const.tile([128, N_BLOCKS, BATCH], f32, tag="y_sb")

            # --- x prep: load, cast, transpose (PE) ---
            nc.sync.dma_start(x_sb[:], x_d[:])
            nc.vector.tensor_copy(x_bf[:], x_sb[:])
            for half in range(2):
                xp = psum.tile([128, 16, BATCH], bf16, tag="xp", bufs=2)
                for c in range(16):
                    g = half * 16 + c
                    nc.tensor.transpose(
                        xp[:, c, :],
                        x_bf[:, g * GROUP : (g + 1) * GROUP],
                        ident[:BATCH, :BATCH],
                    )
                nc.vector.tensor_copy(xT[:, half * 16 : (half + 1) * 16, :], xp[:])

            # --- signsT chunk DMAs: host image is [128, chunk-major (g, o)] so
            # each partition's chunk slice is contiguous (32*w bytes) ---
            s_chunks = []
            off = 0
            for c, w in enumerate(_chunk_widths()):
                sc = const.tile([128, N_GROUPS, w], sdt, tag=f"sT{c}")
                half = N_GROUPS // 2 * w
                for q in range(2):  # 2 queue-parallel halves per chunk
                    nc.sync.dma_start(
                        sc[:, q * 16 : (q + 1) * 16, :],
                        sT_d[:, off + q * half : off + (q + 1) * half].rearrange(
                            "p (g o) -> p g o", g=16
                        ),
                    )
                off += N_GROUPS * w
                s_chunks.append(sc)

            # --- scales (batched: 2 DMAs) ---
            nc.sync.dma_start(
                scales_sb[:, 0:10, :],
                scales_d[0 : 10 * 128].rearrange("(blk p) g -> p blk g", p=128),
            )
            nc.sync.dma_start(scales_sb[:96, 10, :], scales_d[10 * 128 : O_SHARD])

            # --- per block: 32 matmuls (g-major contiguous psum writes), then
            # 2 scale-TTs that write b-major (strided out) into tmp so the
            # final reduce over g is a contiguous innermost reduce ---
            for b in range(N_BLOCKS):
                r = min(128, O_SHARD - b * 128)
                sc = s_chunks[b // 2]
                oc = (b % 2) * 128
                ps0 = psum.tile([128, 16, BATCH], f32, tag="ps0", bufs=2)
                ps1 = psum.tile([128, 16, BATCH], f32, tag="ps1", bufs=2)
                ph = [ps0, ps1]
                for g in range(N_GROUPS):
                    nc.tensor.matmul(
                        ph[g // 16][:r, g % 16, :],
                        sc[:, g, oc : oc + r],
                        xT[:, g, :],
                        start=True,
                        stop=True,
                    )
                tmp = tmp_p.tile([128, BATCH, N_GROUPS], f32, tag="tmp")
                for h in range(2):
                    nc.vector.tensor_tensor(
                        tmp[:r, :, h * 16 : (h + 1) * 16].transpose([0, 2, 1]),
                        ph[h][:r],
                        scales_sb[:r, b, h * 16 : (h + 1) * 16].to_broadcast(
                            [r, 16, BATCH]
                        ),
                        mybir.AluOpType.mult,
                    )
                nc.vector.tensor_reduce(
                    y_sb[:r, b, :],
                    tmp[:r],
                    axis=mybir.AxisListType.X,
                    op=mybir.AluOpType.add,
                )

            # --- y out: [128, 11, 32] -> yT [1376, 32] ---
            nc.sync.dma_start(
                y_d[0 : 10 * 128].rearrange("(blk p) b -> p blk b", p=128),
                y_sb[:, 0:10, :],
            )
            nc.sync.dma_start(y_d[10 * 128 : O_SHARD], y_sb[:96, 10, :])
    nc.finalize()
    return nc


def _pack_signs(signs_shard):
    """[O_SHARD, IN_F] +/-1 -> SBUF image [128, IMG_F]: per partition p, the
    free axis is [chunk][g][o_local] so each chunk DMA reads contiguously."""
    import ml_dtypes

    np_dt = ml_dtypes.float8_e4m3 if SIGN_DT == "fp8" else ml_dtypes.bfloat16
    sT = signs_shard.T.astype(np.float32)  # [IN_F, O_SHARD]
    img = np.empty((128, IMG_F), dtype=np_dt)
    off = 0
    o0 = 0
    for w in _chunk_widths():
        sub = sT[:, o0 : o0 + w].reshape(N_GROUPS, 128, w)
        img[:, off : off + N_GROUPS * w] = (
            sub.transpose(1, 0, 2).reshape(128, N_GROUPS * w).astype(np_dt)
        )
        off += N_GROUPS * w
        o0 += w
    return img


def _shard_inputs(x, scales, signs):
    scales_r = scales.reshape(OUT_F, N_GROUPS)
    x32 = np.ascontiguousarray(x, dtype=np.float32)
    in_maps = []
    for c in range(N_CORES):
        lo, hi = c * O_SHARD, (c + 1) * O_SHARD
        in_maps.append(
            {
                "x": x32,
                "signsT": _pack_signs(signs[lo:hi]),
                "scales": np.ascontiguousarray(scales_r[lo:hi], dtype=np.float32),
            }
        )
    return in_maps


def _run(x, scales, signs, trace=False, tmpdir=None):
    from concourse import bass_utils

    if not _nc_cache:
        _nc_cache.append(build_nc())
    nc = _nc_cache[0]
    in_maps = _shard_inputs(x, scales, signs)
    res = bass_utils.run_bass_kernel_spmd(
        nc, in_maps, list(range(N_CORES)), trace=trace, tmpdir=tmpdir
    )
    out = np.concatenate(
        [np.asarray(res.results[i]["y"]).T for i in range(N_CORES)], axis=1
    )
    return np.ascontiguousarray(out).astype(np.float32), res


def kernel(x, scales, signs):
    out, _ = _run(x, scales, signs)
    return out
